# revision 21
# baseline (speedup 1.0000x reference)
"""Trainium2 Bass kernel for causal multi-head attention block (B=8, S=1024, D=1024, H=16).

Sharding: pure batch data-parallelism - one batch element per NeuronCore (B=8, 8 cores).
Each core runs the full transformer block on its [S, D] slice; no collectives.

v2 (vs v1): all matmul operands in bf16 (dodges the fp32-HIGH power throttle that
duty-cycled the PE to 50% for ~150us, halves weight DMA + LDWEIGHTS), Vp pair layout
[V_even(64) | padm | padm | V_odd(64) | pad2] so odd-head ctx lands directly on PSUM
partitions 64..127 (no gpsimd partition-shift copies), engine rebalance (exps +
proj-bias + LN scale on ACT; PSUM casts, V-scatter, residual, epilogue on DVE;
SBUF-only causal masks and ctx->QT copies on Pool/gpsimd, which cannot touch PSUM),
and software pipelining (scores run 2 groups ahead of ctx; qc0 epilogue and
out-proj of s-blocks 0..3 overlap qc1 attention).

Per-core algorithm:
  - Host passes x^T and all W^T pre-strided bf16 in SBUF partition layout [p, db, o].
  - Q^T/K^T computed as [o, s] via matmul(lhsT=W^T block, rhs=x^T); per-head [dk, S]
    slices feed the scores matmul directly. Bias added on Pool.
  - V natural [s, o], scattered into pair blocks with padm zeroing on DVE; the padm
    columns double as softmax-denominator (s0) rows of the ctx matmul.
  - scoresT[k, q] = matmul(lhsT=K^T slice, rhs=Q^T slice); u = exp(0.125*scores) on
    ACT (bf16); causal 0/1 mask on diagonal blocks on Pool. Padded keys contribute 0
    via zeroed V'/padm rows, so masked entries are exactly 0 in ctx and s0.
  - ctx even head: 65-col lhsT window -> ctx at psum parts 0..63, s0 at 64.
    ctx odd head: 128-col window (start pair_base+2) -> s0 at part 63, ctx at
    64..127. One DVE cast PSUM->bf16 staging, shift-free Pool copies into the QT
    overlay, s0 rows moved cross-partition into the bf16 normalizer table by
    SBUF->SBUF DMA.
  - Rows with empty causal window (s0 == 0) match reference softmax(-1e9*ones) =
    uniform over ALL 1024 keys: ctx = (ctx_u + bad*sumV)/(s0 + 1024*bad).
  - Per-(head,q) normalizers broadcast across partitions with a selector matmul.
  - Out projection + residual (Pool) + LayerNorm (bn_stats on DVE, scale on ACT).
    gamma/beta applied on host (exact no-op for the reference's ones/zeros).
"""

import sys

import numpy as np

if "/opt/trn_rl_repo" not in sys.path:
    sys.path.insert(0, "/opt/trn_rl_repo")

S = 1024
D = 1024
H = 16
DK = 64
P = 128
DB = D // P  # 8 d-blocks
SB = S // P  # 8 s-blocks
SCALE = 0.125  # 1/sqrt(64)
EPS = 1e-5
N_CORES = 8
PRW = 132  # Vp pair-block width: [V_even(64) | padm | padm | V_odd(64) | pad(2)]

_built = None


def _kbs(qc):
    """k-block pairs needed for q-chunk qc (q in [qc*512, qc*512+512))."""
    return [(0, 1), (2, 3)] if qc == 0 else [(0, 1), (2, 3), (4, 5), (6, 7)]


def _vs(kb, qc):
    """first causal q column within the 512-wide chunk for k-block kb."""
    return max(0, kb * P - qc * 512)


def _build():
    import concourse.mybir as mybir
    import concourse.tile as tile
    from concourse import bacc

    f32 = mybir.dt.float32
    bf16 = mybir.dt.bfloat16
    Alu = mybir.AluOpType
    Act = mybir.ActivationFunctionType

    nc = bacc.Bacc()

    # ---- DRAM I/O (pre-strided [p, db, cols] contiguous layouts from host) ----
    xt_d = nc.dram_tensor("xt", [P, DB, S], bf16, kind="ExternalInput")
    xr_d = nc.dram_tensor("xr", [S, D], f32, kind="ExternalInput")  # x + bo_row
    w_d = {
        n: nc.dram_tensor(n, [P, DB, D], bf16, kind="ExternalInput")
        for n in ("wq", "wk", "wv", "wo")
    }
    bqs_d = nc.dram_tensor("bqs", [P, DB], f32, kind="ExternalInput")
    bks_d = nc.dram_tensor("bks", [P, DB], f32, kind="ExternalInput")
    padm_d = nc.dram_tensor("padm", [P, SB], f32, kind="ExternalInput")  # 1 valid/0 pad
    sumv_d = nc.dram_tensor("sumv", [P, DB], f32, kind="ExternalInput")
    pairsel_d = nc.dram_tensor("pairsel", [2 * H, SB * P], bf16, kind="ExternalInput")
    # causal masking via matmul: scores[diag] += negi.T @ ltri = -800 * [k > q];
    # exp(0.125*(s-800)) underflows to exactly 0 in bf16
    negi_d = nc.dram_tensor("negi", [P, P], bf16, kind="ExternalInput")
    ltri_d = nc.dram_tensor("ltri", [P, P], bf16, kind="ExternalInput")
    out_d = nc.dram_tensor("out", [S, D], f32, kind="ExternalOutput")

    with tile.TileContext(nc) as tc:
        with (
            tc.tile_pool(name="singles", bufs=1) as singles,
            tc.tile_pool(name="wpool", bufs=3) as wpool,
            tc.tile_pool(name="big", bufs=1) as big,
            tc.tile_pool(name="upool", bufs=6) as upool,
            tc.tile_pool(name="xres", bufs=4) as xres_pool,
            tc.tile_pool(name="stg", bufs=3) as stg_pool,
            tc.tile_pool(name="small", bufs=2) as small,
            tc.tile_pool(name="ps", bufs=4, space="PSUM") as pspool,
        ):
            # ---- weights + x^T loads (wq first so Q matmuls start early) ----
            wsb = {}
            for n in ("wq", "wk", "wv"):
                w = wpool.tile([P, DB, D], bf16, tag="w", name=n)
                nc.sync.dma_start(w[:, 0:4, :], w_d[n][:, 0:4, :])
                if n == "wq":
                    xt = big.tile([P, DB, S], bf16, tag="xt")
                    for i in range(4):
                        nc.sync.dma_start(
                            xt[:, 2 * i : 2 * i + 2, :], xt_d[:, 2 * i : 2 * i + 2, :]
                        )
                nc.sync.dma_start(w[:, 4:8, :], w_d[n][:, 4:8, :])
                wsb[n] = w

            QT = big.tile([P, DB, S], bf16, tag="qt")  # later overlaid with ctx^T
            KT = big.tile([P, DB, S], bf16, tag="kt")
            Vp = big.tile([P, SB, SB * PRW], bf16, tag="vp")

            # ---- constants / singles ----
            bq_sb = singles.tile([P, DB], f32)
            nc.sync.dma_start(bq_sb[:], bqs_d[:, :])
            bk_sb = singles.tile([P, DB], f32)
            nc.sync.dma_start(bk_sb[:], bks_d[:, :])
            padm_sb = singles.tile([P, SB], f32)
            nc.sync.dma_start(padm_sb[:], padm_d[:, :])
            negi_sb = singles.tile([P, P], bf16)
            nc.sync.dma_start(negi_sb[:], negi_d[:, :])
            ltri_sb = singles.tile([P, P], bf16)
            nc.sync.dma_start(ltri_sb[:], ltri_d[:, :])
            pairsel = singles.tile([2 * H, SB * P], bf16)
            nc.sync.dma_start(pairsel[:], pairsel_d[:, :])
            sumv_all = singles.tile([P, DB], f32)
            nc.sync.dma_start(sumv_all[:], sumv_d[:, :])
            eps_sb = singles.tile([P, 1], f32)
            nc.vector.memset(eps_sb[:], EPS)

            # normalizer table (bf16, K=32 selector rhs): cols 0:S s0 (later
            # recip), S:2S bad*1024; rows 16..31 stay zero.
            tab = singles.tile([2 * H, 2 * S], bf16)
            nc.vector.memset(tab[:], 0.0)

            # ============ Phase 1: Q/K projections ============
            for wname, dst, bias_sb in (("wq", QT, bq_sb), ("wk", KT, bk_sb)):
                w = wsb[wname]
                for ob in range(DB):
                    ps = pspool.tile([P, 2, 512], f32, tag="mm")
                    for sc in range(2):
                        for db in range(DB):
                            nc.tensor.matmul(
                                ps[:, sc, :],
                                lhsT=w[:, db, ob * P : (ob + 1) * P],
                                rhs=xt[:, db, sc * 512 : (sc + 1) * 512],
                                start=(db == 0),
                                stop=(db == DB - 1),
                            )
                    # per-partition bias add (o on partitions) on DVE (phase-1
                    # DVE is idle; keeps ACT free for the first exps)
                    nc.vector.tensor_scalar(
                        dst[:, ob, :],
                        ps[:].rearrange("p a b -> p (a b)"),
                        bias_sb[:, ob : ob + 1],
                        None,
                        op0=Alu.add,
                    )

            # wo reuses wq's buffer; DMA overlaps V-proj + attention
            wo = wpool.tile([P, DB, D], bf16, tag="w", name="wo")
            nc.sync.dma_start(wo[:, 0:4, :], w_d["wo"][:, 0:4, :])
            nc.sync.dma_start(wo[:, 4:8, :], w_d["wo"][:, 4:8, :])

            # padm columns of the Vp pair blocks (cols 64,65 of each 130 block)
            vpair = Vp[:].rearrange("p sb (pr c) -> p sb pr c", c=PRW)
            nc.vector.tensor_copy(
                vpair[:, :, :, DK : DK + 2],
                padm_sb.unsqueeze(2).unsqueeze(3).to_broadcast([P, SB, DB, 2]),
            )

            # ============ Phase 1b: V projection ============
            wv = wsb["wv"]
            for oc in range(2):
                for sbi in range(0, SB, 2):
                    ps = pspool.tile([P, 2, 512], f32, tag="mm")
                    for si in range(2):
                        sb = sbi + si
                        for db in range(DB):
                            nc.tensor.matmul(
                                ps[:, si, :],
                                lhsT=xt[:, db, sb * P : (sb + 1) * P],
                                rhs=wv[:, db, oc * 512 : (oc + 1) * 512],
                                start=(db == 0),
                                stop=(db == DB - 1),
                            )
                    for si in range(2):
                        sb = sbi + si
                        # scatter into pair blocks (even heads lo at col 0, odd
                        # heads hi at col 66), zeroing padded keys via padm (DVE).
                        # The 66-col sub-split works because PRW=132=2*66.
                        psv = ps[:, si, :].rearrange(
                            "p (h par c) -> p h par c", c=DK, par=2
                        )
                        vdst = vpair[:, sb, 4 * oc : 4 * oc + 4, :].rearrange(
                            "p pr (par c2) -> p pr par c2", c2=PRW // 2
                        )
                        nc.vector.tensor_scalar(
                            vdst[:, :, :, 0:DK],
                            psv[:, :, :, :],
                            padm_sb[:, sb : sb + 1],
                            None,
                            op0=Alu.mult,
                        )

            # ============ Phase 2: attention ============
            groups = [(0, hb) for hb in range(8)] + [(1, hb) for hb in range(8)]
            uts = {}

            def scores(qc, hb):
                us = (
                    upool.tile([P, SB, 512], bf16, tag="u", name=f"u{qc}_{hb}_0"),
                    upool.tile([P, SB, 512], bf16, tag="u", name=f"u{qc}_{hb}_1"),
                )
                uts[(qc, hb)] = us
                for kb0, kb1 in _kbs(qc):
                    vs = _vs(kb0, qc)
                    for par in range(2):
                        hp = DK * par
                        ut = us[par]
                        ps = pspool.tile([P, 2, 512], f32, tag="mm")
                        for i, kb in enumerate((kb0, kb1)):
                            diag = kb * P >= qc * 512
                            nc.tensor.matmul(
                                ps[:, i, vs:512],
                                lhsT=KT[hp : hp + DK, hb, kb * P : (kb + 1) * P],
                                rhs=QT[
                                    hp : hp + DK, hb, qc * 512 + vs : qc * 512 + 512
                                ],
                                start=True,
                                stop=not diag,
                            )
                            if diag:
                                # causal mask: scores[k, dvs+q'] += -800*[k > q']
                                dvs = _vs(kb, qc)
                                nc.tensor.matmul(
                                    ps[:, i, dvs : dvs + P],
                                    lhsT=negi_sb[:],
                                    rhs=ltri_sb[:],
                                    start=False,
                                    stop=True,
                                )
                        # u = exp(0.125*scores); both k-blocks in one ACT op;
                        # masked entries underflow to exactly 0
                        nc.scalar.activation(
                            ut[:, kb0 : kb0 + 2, vs:512],
                            ps[:, :, vs:512],
                            Act.Exp,
                            scale=SCALE,
                        )

            def ctx(qc, hb):
                qcs = slice(qc * 512, (qc + 1) * 512)
                us = uts.pop((qc, hb))
                ps = pspool.tile([P, 2, 512], f32, tag="mm")
                klist = [kb for pr in _kbs(qc) for kb in pr]
                base = hb * PRW
                for par in range(2):
                    for i, kb in enumerate(klist):
                        vs = _vs(kb, qc)
                        if par == 0:
                            # even head: V at psum parts 0..63, s0 at 64
                            out_ap = ps[0:65, 0, vs:512]
                            lhsT = Vp[:, kb, base : base + DK + 1]
                        else:
                            # odd head: junk at 0..62, s0 at 63, V at 64..127
                            out_ap = ps[:, 1, vs:512]
                            lhsT = Vp[:, kb, base + 2 : base + DK + 2 + DK]
                        nc.tensor.matmul(
                            out_ap,
                            lhsT=lhsT,
                            rhs=us[par][:, kb, vs:512],
                            start=(i == 0),
                            stop=(i == len(klist) - 1),
                        )
                # one DVE cast of both banks (incl. junk partitions) to bf16
                stg = stg_pool.tile([P, 2, 512], bf16, tag="stg")
                nc.vector.tensor_copy(stg[:, :, :], ps[:, :, :])
                # ctx halves -> QT overlay (shift-free, DVE 2x bf16 SBUF->SBUF)
                nc.vector.tensor_copy(QT[0:DK, hb, qcs], stg[0:DK, 0, :])
                nc.vector.tensor_copy(QT[DK:P, hb, qcs], stg[DK:P, 1, :])
                # s0 rows -> normalizer table (cross-partition SBUF->SBUF DMA)
                nc.sync.dma_start(
                    tab[2 * hb : 2 * hb + 1, qcs], stg[DK : DK + 1, 0, :]
                )
                nc.sync.dma_start(
                    tab[2 * hb + 1 : 2 * hb + 2, qcs], stg[DK - 1 : DK, 1, :]
                )

            def recip(qc):
                qcs = slice(qc * 512, (qc + 1) * 512)
                q2s = slice(S + qc * 512, S + (qc + 1) * 512)
                T0 = tab[0:H, qcs]
                T2 = tab[0:H, q2s]
                nc.vector.tensor_scalar(
                    T2, T0, 1e-9, 1024.0, op0=Alu.is_le, op1=Alu.mult
                )
                nc.vector.tensor_tensor(T0, T0, T2, Alu.add)
                with nc.allow_low_precision(
                    reason="bf16 softmax normalizer; ~0.4% uniform scale error is "
                    "far below the output tolerance"
                ):
                    nc.vector.reciprocal(T0, T0)

            def sel_epi(qc, hb):
                qcs = slice(qc * 512, (qc + 1) * 512)
                bc = pspool.tile([P, 2, 512], f32, tag="mm", name="bc")
                nc.tensor.matmul(
                    bc[:, 0, :],
                    lhsT=pairsel[:, hb * P : (hb + 1) * P],
                    rhs=tab[:, S + qc * 512 : S + (qc + 1) * 512],
                    start=True,
                    stop=True,
                )
                nc.tensor.matmul(
                    bc[:, 1, :],
                    lhsT=pairsel[:, hb * P : (hb + 1) * P],
                    rhs=tab[:, qc * 512 : (qc + 1) * 512],
                    start=True,
                    stop=True,
                )
                # ctx = (ctx_u + bad1024 * sumV/1024) * recip
                nc.vector.scalar_tensor_tensor(
                    QT[:, hb, qcs],
                    bc[:, 0, :],
                    sumv_all[:, hb : hb + 1],
                    QT[:, hb, qcs],
                    op0=Alu.mult,
                    op1=Alu.add,
                )
                nc.vector.tensor_tensor(
                    QT[:, hb, qcs], QT[:, hb, qcs], bc[:, 1, :], Alu.mult
                )

            def outproj(sb):
                xres = xres_pool.tile([P, D], f32, tag="xres")
                nc.sync.dma_start(xres[:], xr_d[sb * P : (sb + 1) * P, :])
                ps = pspool.tile([P, 2, 512], f32, tag="mm")
                for oc in range(2):
                    for db in range(DB):
                        nc.tensor.matmul(
                            ps[:, oc, :],
                            lhsT=QT[:, db, sb * P : (sb + 1) * P],
                            rhs=wo[:, db, oc * 512 : (oc + 1) * 512],
                            start=(db == 0),
                            stop=(db == DB - 1),
                        )
                # residual add on DVE (PSUM reader)
                nc.vector.tensor_tensor(
                    xres[:, :], ps[:].rearrange("p a b -> p (a b)"), xres[:, :], Alu.add
                )
                # LayerNorm over free dim (1024) via bn_stats (2 subgroups of 512)
                stats = small.tile([P, 2, 6], f32, tag="stats")
                nc.vector.bn_stats(stats[:, 0, :], xres[:, 0:512])
                nc.vector.bn_stats(stats[:, 1, :], xres[:, 512:1024])
                mv = small.tile([P, 2], f32, tag="mv")
                nc.vector.bn_aggr(mv[:], stats[:])
                rstd = small.tile([P, 2], f32, tag="rstd")
                nc.scalar.activation(
                    rstd[:, 0:1], mv[:, 1:2], Act.Sqrt, bias=eps_sb[:], scale=1.0
                )
                nc.vector.reciprocal(rstd[:, 0:1], rstd[:, 0:1])
                # -mu * rstd
                nc.vector.tensor_scalar(
                    rstd[:, 1:2],
                    mv[:, 0:1],
                    -1.0,
                    rstd[:, 0:1],
                    op0=Alu.mult,
                    op1=Alu.mult,
                )
                # out = x*rstd - mu*rstd on ACT, in place
                nc.scalar.activation(
                    xres[:, :],
                    xres[:, :],
                    Act.Identity,
                    bias=rstd[:, 1:2],
                    scale=rstd[:, 0:1],
                )
                nc.sync.dma_start(out_d[sb * P : (sb + 1) * P, :], xres[:, :])

            # pipeline: scores run 2 groups ahead of ctx; qc0 epilogue and the
            # first out-proj half overlap qc1 attention
            scores(*groups[0])
            scores(*groups[1])
            for i, (qc, hb) in enumerate(groups):
                if i + 2 < len(groups):
                    scores(*groups[i + 2])
                ctx(qc, hb)
                if i == 7:
                    recip(0)
                if i >= 8:
                    sel_epi(0, i - 8)
            for sb in range(4):
                outproj(sb)
            recip(1)
            for hb in range(8):
                sel_epi(1, hb)
            for sb in range(4, 8):
                outproj(sb)

    nc.compile()
    return nc


def kernel(
    history_items,
    sequence_mask,
    Wq,
    bq,
    Wk,
    bk,
    Wv,
    bv,
    Wo,
    bo,
    ln_gamma,
    ln_beta,
):
    import ml_dtypes

    from concourse.bass_utils import run_bass_kernel_spmd

    bf16 = ml_dtypes.bfloat16

    global _built
    if _built is None:
        _built = _build()
    nc = _built

    x = np.asarray(history_items, dtype=np.float32)
    mask = np.asarray(sequence_mask)
    f = lambda a: np.ascontiguousarray(np.asarray(a, dtype=np.float32))
    fb = lambda a: np.ascontiguousarray(np.asarray(a, dtype=np.float32).astype(bf16))

    common = {}
    for wname, W in (("wq", Wq), ("wk", Wk), ("wv", Wv), ("wo", Wo)):
        WT = np.asarray(W, dtype=np.float32).T  # [d_in, d_out]
        common[wname] = fb(WT.reshape(DB, P, D).transpose(1, 0, 2))
    common["bqs"] = f(np.asarray(bq).reshape(DB, P).T)
    common["bks"] = f(np.asarray(bk).reshape(DB, P).T)
    k_idx = np.arange(2 * H)[:, None]
    hb_idx = np.repeat(np.arange(SB), P)[None, :]
    c1_idx = np.tile((np.arange(P) >= 64).astype(np.int64), SB)[None, :]
    common["pairsel"] = fb((k_idx == 2 * hb_idx + c1_idx).astype(np.float32))
    common["negi"] = fb(-800.0 * np.eye(P, dtype=np.float32))
    common["ltri"] = fb(
        np.where(np.arange(P)[:, None] > np.arange(P)[None, :], 1.0, 0.0)
    )
    # attn-output bias bv contributes bv @ Wo.T (constant over s) -> fold into residual
    bo_row = (
        np.asarray(bo, dtype=np.float64)
        + np.asarray(bv, dtype=np.float64) @ np.asarray(Wo, dtype=np.float64).T
    ).astype(np.float32)

    in_maps = []
    for b in range(N_CORES):
        xT = x[b].T.reshape(DB, P, S).transpose(1, 0, 2)  # [p, db, s]
        pm = (mask[b] != 0).astype(np.float32)
        sx = x[b].astype(np.float64).sum(axis=0)
        sumv = ((sx @ np.asarray(Wv, dtype=np.float64).T) / 1024.0).astype(np.float32)
        in_maps.append(
            {
                **common,
                "xt": fb(xT),
                "xr": f(x[b] + bo_row[None, :]),
                "padm": f(pm.reshape(SB, P).T),
                "sumv": f(sumv.reshape(DB, P).T),
            }
        )

    r = run_bass_kernel_spmd(nc, in_maps, core_ids=list(range(N_CORES)))
    out = np.stack([res["out"] for res in r.results]).astype(np.float32)

    g = np.asarray(ln_gamma, dtype=np.float32)
    be = np.asarray(ln_beta, dtype=np.float32)
    out = out * g[None, None, :] + be[None, None, :]
    return out.astype(np.float32)
